# revision 2
# baseline (speedup 1.0000x reference)
"""MoE group-limited routing gate (DeepSeek-style) on 8 Trainium2 NeuronCores.

Computation (per token t over E=256 experts, D=7168 features):
    logits = x @ weight.T                      [T, E]
    group-limited top-k: 8 groups of 32 experts, keep top-4 groups by
    group-max, then top-8 experts among kept groups.
    weights = sigmoid(logits[sel]) normalized to sum 1, * 2.5
Returns (weights [T,8] f32, indices [T,8] int32) like the reference.

Strategy: data-parallel over tokens, 2048 tokens/core, gate weight
replicated.  x is pre-transposed on host to [D, T] so the contraction dim
lands on SBUF partitions.  Matmul precision options:
  - "fp16x3": x and w are split on host into fp16 (hi, lo*2^11) pairs;
    logits = hi@hi + 2^-11*(hi@lo2 + lo2@hi).  fp16 products are exact in
    the f32 PSUM accumulator, so the result carries ~f32-grade accuracy
    (~1e-6) at 3 bf16-rate passes, and index flips vs the f32 reference
    are ~zero.  DMA bytes are unchanged (2+2 B/elem).
  - "f32r": single-pass full-rate fp32 (13-bit-ish products) — fastest,
    but logit error ~2e-4 flips ~0.1% of top-k indices.
Top-k uses the DVE native max/max_index (top-8 sorted) instructions; the
group top-4 uses a threshold trick (4th-largest group-max) since sigmoid
is monotone and masking is additive on logits.
"""

import numpy as np
from contextlib import ExitStack

import concourse.bacc as bacc
import concourse.tile as tile
from concourse import mybir
from concourse.bass_utils import run_bass_kernel_spmd

N_CORES = 8
T_FULL = 16384
D = 7168
E = 256
G = 8            # expert groups
EPG = E // G     # experts per group = 32
TOPK = 8
TOPK_GROUPS = 4
ROUTE_SCALE = 2.5

P = 128
T = T_FULL // N_CORES       # 2048 tokens per core
KC = D // P                 # 56 contraction chunks
TB = 256                    # tokens per block
NB = T // TB                # 8 blocks
TPB = TB // P               # 2 token-tiles per block
KQ = 4                      # x DMA splits per block (finer-grained deps)
KCQ = KC // KQ              # 14 k-chunks per split
WQ = 8                      # weight DMA splits
WCQ = KC // WQ              # 7 k-chunks per split
NEG = -1.0e30
LO_SCALE = 2.0 ** 11        # host scales the fp16 lo term by this
PRECISION = "fp16x3"        # "fp16x3" | "f32r"

_CACHE = {}


def _emit_topk(nc, sc_pool, out_pool, scores, wout, iout, t0):
    """Group-limited top-k + normalize on a [128, 256] f32 logits tile."""
    f32 = mybir.dt.float32
    scores_g = scores.rearrange("p (g e) -> p g e", g=G)
    glog = sc_pool.tile([P, G], f32)
    nc.vector.reduce_max(out=glog, in_=scores_g, axis=mybir.AxisListType.X)
    gsort = sc_pool.tile([P, G], f32)
    nc.vector.max(out=gsort, in_=glog)
    # additive mask: 0 for kept groups (>= 4th-largest), -1e30 otherwise
    maskadd = sc_pool.tile([P, G], f32)
    nc.vector.tensor_scalar(
        out=maskadd,
        in0=glog,
        scalar1=gsort[:, TOPK_GROUPS - 1:TOPK_GROUPS],
        scalar2=NEG,
        op0=mybir.AluOpType.is_lt,
        op1=mybir.AluOpType.mult,
    )
    masked = sc_pool.tile([P, E], f32)
    nc.vector.tensor_add(
        masked.rearrange("p (g e) -> p g e", g=G),
        scores_g,
        maskadd.to_broadcast([P, G, EPG]),
    )
    top8 = sc_pool.tile([P, TOPK], f32)
    nc.vector.max(out=top8, in_=masked)
    idx = out_pool.tile([P, TOPK], mybir.dt.uint32)
    nc.vector.max_index(out=idx, in_max=top8, in_values=masked)
    sig = sc_pool.tile([P, TOPK], f32)
    nc.scalar.activation(
        out=sig, in_=top8, func=mybir.ActivationFunctionType.Sigmoid
    )
    ssum = sc_pool.tile([P, 1], f32)
    nc.vector.reduce_sum(out=ssum, in_=sig, axis=mybir.AxisListType.X)
    rec = sc_pool.tile([P, 1], f32)
    nc.vector.reciprocal(out=rec, in_=ssum)
    wres = out_pool.tile([P, TOPK], f32)
    nc.vector.tensor_scalar(
        out=wres,
        in0=sig,
        scalar1=rec[:, 0:1],
        scalar2=ROUTE_SCALE,
        op0=mybir.AluOpType.mult,
        op1=mybir.AluOpType.mult,
    )
    # outputs ride the SWDGE ring so the tiny writes never stall the
    # HWDGE ring that streams x
    nc.gpsimd.dma_start(out=wout[t0:t0 + P, :], in_=wres)
    nc.gpsimd.dma_start(out=iout[t0:t0 + P, :], in_=idx)


def _build_f32r():
    nc = bacc.Bacc("TRN2", target_bir_lowering=False, debug=False, num_devices=N_CORES)
    f32 = mybir.dt.float32
    f32r = mybir.dt.float32r
    xt = nc.dram_tensor("xt", [D, T], f32r, kind="ExternalInput").ap()
    wt = nc.dram_tensor("wt", [D, E], f32r, kind="ExternalInput").ap()
    wout = nc.dram_tensor("w_out", [T, TOPK], f32, kind="ExternalOutput").ap()
    iout = nc.dram_tensor("i_out", [T, TOPK], mybir.dt.uint32, kind="ExternalOutput").ap()

    xt_r = xt.rearrange("(k p) t -> p k t", p=P)
    wt_r = wt.rearrange("(k p) e -> p k e", p=P)

    with tile.TileContext(nc) as tc, ExitStack() as ctx:
        wt_pool = ctx.enter_context(tc.tile_pool(name="wt", bufs=1))
        xt_pool = ctx.enter_context(tc.tile_pool(name="xt", bufs=2))
        psum_pool = ctx.enter_context(tc.tile_pool(name="psum", bufs=4, space="PSUM"))
        sc_pool = ctx.enter_context(tc.tile_pool(name="scratch", bufs=3))
        out_pool = ctx.enter_context(tc.tile_pool(name="outs", bufs=4))

        wt_sb = []
        for q in range(WQ):
            wtile = wt_pool.tile([P, WCQ, E], f32r, tag=f"wt{q}")
            nc.sync.dma_start(out=wtile, in_=wt_r[:, q * WCQ:(q + 1) * WCQ, :])
            wt_sb.append(wtile)

        for b in range(NB):
            xq = []
            for q in range(KQ):
                xtile = xt_pool.tile([P, KCQ, TB], f32r, tag=f"xt{q}")
                nc.sync.dma_start(
                    out=xtile,
                    in_=xt_r[:, q * KCQ:(q + 1) * KCQ, b * TB:(b + 1) * TB],
                )
                xq.append(xtile)
            for j in range(TPB):
                psum = psum_pool.tile([P, E], f32)
                for k in range(KC):
                    lhsT = xq[k // KCQ][:, k % KCQ, j * P:(j + 1) * P]
                    rhs = wt_sb[k // WCQ][:, k % WCQ, :]
                    nc.tensor.matmul(psum, lhsT, rhs, start=(k == 0), stop=(k == KC - 1))
                _emit_topk(nc, sc_pool, out_pool, psum, wout, iout, b * TB + j * P)
    nc.compile()
    return nc


def _build_fp16x3():
    nc = bacc.Bacc("TRN2", target_bir_lowering=False, debug=False, num_devices=N_CORES)
    f32 = mybir.dt.float32
    f16 = mybir.dt.float16
    xh = nc.dram_tensor("xh", [D, T], f16, kind="ExternalInput").ap()
    xl = nc.dram_tensor("xl", [D, T], f16, kind="ExternalInput").ap()
    wh = nc.dram_tensor("wh", [D, E], f16, kind="ExternalInput").ap()
    wl = nc.dram_tensor("wl", [D, E], f16, kind="ExternalInput").ap()
    wout = nc.dram_tensor("w_out", [T, TOPK], f32, kind="ExternalOutput").ap()
    iout = nc.dram_tensor("i_out", [T, TOPK], mybir.dt.uint32, kind="ExternalOutput").ap()

    xh_r = xh.rearrange("(k p) t -> p k t", p=P)
    xl_r = xl.rearrange("(k p) t -> p k t", p=P)
    wh_r = wh.rearrange("(k p) e -> p k e", p=P)
    wl_r = wl.rearrange("(k p) e -> p k e", p=P)

    with tile.TileContext(nc) as tc, ExitStack() as ctx:
        wt_pool = ctx.enter_context(tc.tile_pool(name="wt", bufs=1))
        xt_pool = ctx.enter_context(tc.tile_pool(name="xt", bufs=2))
        # 4+4 slots = all 8 PSUM banks: block b's accumulators coexist with
        # block b-1's (whose xh@wl half is deferred one block, see below)
        psA_pool = ctx.enter_context(tc.tile_pool(name="psA", bufs=4, space="PSUM"))
        psB_pool = ctx.enter_context(tc.tile_pool(name="psB", bufs=4, space="PSUM"))
        sc_pool = ctx.enter_context(tc.tile_pool(name="scratch", bufs=3))
        out_pool = ctx.enter_context(tc.tile_pool(name="outs", bufs=4))

        # DMA emission order matters: the sync HWDGE ring drains FIFO, so
        # put the first weight quarter + block-0 x tiles up front to get the
        # PE computing within a few us, then stream the rest of the weights.
        def load_w(q, which):
            src, lst, tag = (
                (wh_r, wh_sb, f"wh{q}") if which == "h" else (wl_r, wl_sb, f"wl{q}")
            )
            wtile = wt_pool.tile([P, WCQ, E], f16, tag=tag)
            nc.sync.dma_start(out=wtile, in_=src[:, q * WCQ:(q + 1) * WCQ, :])
            lst.append(wtile)

        def load_x_block(b):
            xh_q, xl_q = [], []
            t_lo, t_hi = b * TB, (b + 1) * TB
            for q in range(KQ):
                xtile = xt_pool.tile([P, KCQ, TB], f16, tag=f"xh{q}")
                nc.sync.dma_start(
                    out=xtile, in_=xh_r[:, q * KCQ:(q + 1) * KCQ, t_lo:t_hi]
                )
                xh_q.append(xtile)
                ltile = xt_pool.tile([P, KCQ, TB], f16, tag=f"xl{q}")
                nc.sync.dma_start(
                    out=ltile, in_=xl_r[:, q * KCQ:(q + 1) * KCQ, t_lo:t_hi]
                )
                xl_q.append(ltile)
            return xh_q, xl_q

        # emission order == consumption order: (wh_q, xh_q) pairs feed pass A,
        # then xl (pass B first half reuses resident wh), then wl quarters.
        wh_sb, wl_sb = [], []
        xh0, xl0 = [], []
        t_hi0 = TB
        for q in range(KQ):
            load_w(2 * q, "h")
            load_w(2 * q + 1, "h")
            xtile = xt_pool.tile([P, KCQ, TB], f16, tag=f"xh{q}")
            nc.sync.dma_start(out=xtile, in_=xh_r[:, q * KCQ:(q + 1) * KCQ, 0:t_hi0])
            xh0.append(xtile)
        for q in range(KQ):
            ltile = xt_pool.tile([P, KCQ, TB], f16, tag=f"xl{q}")
            nc.sync.dma_start(out=ltile, in_=xl_r[:, q * KCQ:(q + 1) * KCQ, 0:t_hi0])
            xl0.append(ltile)
        for q in range(WQ):
            load_w(q, "l")
        blocks = {0: (xh0, xl0)}

        # Per block: pass A (xh@wh) and pass B first half (xl@wh) run with
        # only the early-arriving wh stream; the xh@wl half is deferred one
        # block so nothing on the critical path waits for the wl quarters.
        def flush(state):
            bb, xh_q, psA_list, psB_list = state
            for j in range(TPB):
                js = slice(j * P, (j + 1) * P)
                psumB = psB_list[j]
                for k in range(KC):
                    nc.tensor.matmul(
                        psumB,
                        xh_q[k // KCQ][:, k % KCQ, js],
                        wl_sb[k // WCQ][:, k % WCQ, :],
                        start=False,
                        stop=(k == KC - 1),
                    )
                scores = sc_pool.tile([P, E], f32)
                nc.scalar.activation(
                    out=scores,
                    in_=psumB,
                    func=mybir.ActivationFunctionType.Copy,
                    scale=1.0 / LO_SCALE,
                )
                nc.vector.tensor_add(scores, scores, psA_list[j])
                _emit_topk(nc, sc_pool, out_pool, scores, wout, iout, bb * TB + j * P)

        # Block 0 runs A + B1 only (no wl dependency) and its xh@wl half is
        # flushed right after block 1's A pass, by which time the wl stream
        # has landed.  Blocks >= 1 use the tight per-tile A,B1+B2 structure
        # so tile release (and thus the next block's DMA prefetch) stays a
        # full block ahead.
        pending = None
        for b in range(NB):
            if b not in blocks:
                blocks[b] = load_x_block(b)
            xh_q, xl_q = blocks.pop(b)
            if b == 0:
                psA_list, psB_list = [], []
                for j in range(TPB):
                    js = slice(j * P, (j + 1) * P)
                    psumA = psA_pool.tile([P, E], f32)
                    for k in range(KC):
                        nc.tensor.matmul(
                            psumA,
                            xh_q[k // KCQ][:, k % KCQ, js],
                            wh_sb[k // WCQ][:, k % WCQ, :],
                            start=(k == 0),
                            stop=(k == KC - 1),
                        )
                    psA_list.append(psumA)
                for j in range(TPB):
                    js = slice(j * P, (j + 1) * P)
                    psumB = psB_pool.tile([P, E], f32)
                    for k in range(KC):
                        nc.tensor.matmul(
                            psumB,
                            xl_q[k // KCQ][:, k % KCQ, js],
                            wh_sb[k // WCQ][:, k % WCQ, :],
                            start=(k == 0),
                            stop=False,
                        )
                    psB_list.append(psumB)
                pending = (b, xh_q, psA_list, psB_list)
                continue
            for j in range(TPB):
                js = slice(j * P, (j + 1) * P)
                psumA = psA_pool.tile([P, E], f32)
                for k in range(KC):
                    nc.tensor.matmul(
                        psumA,
                        xh_q[k // KCQ][:, k % KCQ, js],
                        wh_sb[k // WCQ][:, k % WCQ, :],
                        start=(k == 0),
                        stop=(k == KC - 1),
                    )
                if pending is not None:
                    flush(pending)
                    pending = None
                psumB = psB_pool.tile([P, E], f32)
                for i in range(2 * KC):
                    k = i % KC
                    if i < KC:
                        lhsT = xl_q[k // KCQ][:, k % KCQ, js]
                        rhs = wh_sb[k // WCQ][:, k % WCQ, :]
                    else:
                        lhsT = xh_q[k // KCQ][:, k % KCQ, js]
                        rhs = wl_sb[k // WCQ][:, k % WCQ, :]
                    nc.tensor.matmul(
                        psumB, lhsT, rhs, start=(i == 0), stop=(i == 2 * KC - 1)
                    )
                scores = sc_pool.tile([P, E], f32)
                nc.scalar.activation(
                    out=scores,
                    in_=psumB,
                    func=mybir.ActivationFunctionType.Copy,
                    scale=1.0 / LO_SCALE,
                )
                nc.vector.tensor_add(scores, scores, psumA)
                _emit_topk(nc, sc_pool, out_pool, scores, wout, iout, b * TB + j * P)
    nc.compile()
    return nc


def _get_program(precision):
    key = f"nc_{precision}"
    if key not in _CACHE:
        _CACHE[key] = _build_fp16x3() if precision == "fp16x3" else _build_f32r()
    return _CACHE[key]


def _split_f16(a):
    hi = a.astype(np.float16)
    lo = ((a - hi.astype(np.float32)) * np.float32(LO_SCALE)).astype(np.float16)
    return hi, lo


def kernel(x: np.ndarray, weight: np.ndarray, _trace: bool = False, **_kw):
    x = np.asarray(x, dtype=np.float32)
    weight = np.asarray(weight, dtype=np.float32)
    assert x.shape == (T_FULL, D) and weight.shape == (E, D)

    nc = _get_program(PRECISION)
    xt_full = np.ascontiguousarray(x.T)              # [D, T_FULL]
    wt_host = np.ascontiguousarray(weight.T)         # [D, E]
    if PRECISION == "fp16x3":
        xh_full, xl_full = _split_f16(xt_full)
        wh_host, wl_host = _split_f16(wt_host)
        in_maps = [
            {
                "xh": np.ascontiguousarray(xh_full[:, c * T:(c + 1) * T]),
                "xl": np.ascontiguousarray(xl_full[:, c * T:(c + 1) * T]),
                "wh": wh_host,
                "wl": wl_host,
            }
            for c in range(N_CORES)
        ]
    else:
        in_maps = [
            {
                "xt": np.ascontiguousarray(xt_full[:, c * T:(c + 1) * T]),
                "wt": wt_host,
            }
            for c in range(N_CORES)
        ]
    if _trace:
        import tempfile

        res = run_bass_kernel_spmd(
            nc, in_maps, core_ids=list(range(N_CORES)), trace=True,
            tmpdir=tempfile.mkdtemp(prefix="moe_gate_trace_"),
        )
        results = res.results
        _CACHE["last_result"] = {
            "exec_time_ns": res.exec_time_ns,
            "percore": res.mean_exec_time_ns,
            "neff_dir": res.instructions_and_trace[1]
            if res.instructions_and_trace
            else None,
        }
    else:
        res = run_bass_kernel_spmd(nc, in_maps, core_ids=list(range(N_CORES)))
        results = res.results
    w_full = np.concatenate([results[c]["w_out"] for c in range(N_CORES)], axis=0)
    i_full = np.concatenate(
        [results[c]["i_out"].astype(np.int32) for c in range(N_CORES)], axis=0
    )
    return w_full, i_full



# revision 3
# speedup vs baseline: 1.4374x; 1.4374x over previous
"""MoE group-limited routing gate (DeepSeek-style) on 8 Trainium2 NeuronCores.

Computation (per token t over E=256 experts, D=7168 features):
    logits = x @ weight.T                      [T, E]
    group-limited top-k: 8 groups of 32 experts, keep top-4 groups by
    group-max, then top-8 experts among kept groups.
    weights = sigmoid(logits[sel]) normalized to sum 1, * 2.5
Returns (weights [T,8] f32, indices [T,8] int32) like the reference.

Strategy: data-parallel over tokens, 2048 tokens/core, gate weight
replicated.  Matmul precision "fp16fp8":
    logits = xh16 @ wh16  +  2^-17 * (x8 @ wl8 + xl8 @ wh8)
  - main pass: fp16 (11-bit significands, products exact in f32 PSUM)
  - correction: ONE fp8e4m3 DoubleRow pass fusing both residual terms
    (x8 = fp8(xh) cast on-device; xl8 = fp8((x-xh16)*2^11) from host;
    wl8 = fp8((w-wh16)*2^17); wh8 = fp8(w*2^6); descale 2^-17).
    DoubleRow runs fp8 at 2x rate, so the whole correction costs one
    bf16-rate pass -> 2 pass-equivalents total vs fp16x3's 3.
  Host-measured logit err ~1.8e-5 -> idx rel-err ~6e-3 on the graded
  inputs (vs 2e-2 gate).
DMA: x rides the SP HWDGE ring at 3 B/elem (xh16 2B + xl8 1B) in
partition-major contiguous blocks; the small replicated weights ride
the Activation HWDGE ring in parallel so they never serialize the x
stream.  Outputs ride the gpsimd SWDGE ring.
Top-k uses the DVE native max/max_index (top-8 sorted) instructions;
the group top-4 uses a threshold trick (4th-largest group-max) since
sigmoid is monotone and masking is additive on logits.
"""

import numpy as np
import ml_dtypes
from contextlib import ExitStack

import concourse.bacc as bacc
import concourse.tile as tile
from concourse import mybir
from concourse.bass_utils import run_bass_kernel_spmd

N_CORES = 8
T_FULL = 16384
D = 7168
E = 256
G = 8            # expert groups
EPG = E // G     # experts per group = 32
TOPK = 8
TOPK_GROUPS = 4
ROUTE_SCALE = 2.5

P = 128
T = T_FULL // N_CORES       # 2048 tokens per core
KC = D // P                 # 56 contraction chunks
TB = 256                    # tokens per block
NB = T // TB                # 8 blocks
TPB = TB // P               # 2 token-tiles per block
KQ = 4                      # x DMA splits per block
KCQ = KC // KQ              # 14 k-chunks per split
WQ = 8                      # weight DMA splits
WCQ = KC // WQ              # 7 k-chunks per split
NEG = -1.0e30
XL_SCALE = 2.0 ** 11        # xl8 = fp8((x - xh16) * XL_SCALE)
WL_SCALE = 2.0 ** 17        # wl8 = fp8((w - wh16) * WL_SCALE)
WH8_SCALE = 2.0 ** 6        # wh8 = fp8(w * WH8_SCALE); XL_SCALE*WH8_SCALE == WL_SCALE
CORR_DESCALE = 1.0 / WL_SCALE
E4 = ml_dtypes.float8_e4m3
PRECISION = "fp16fp8"       # "fp16fp8" | "fp16x3"

_CACHE = {}


def _emit_topk(nc, sc_pool, out_pool, scores, wout, iout, t0):
    """Group-limited top-k + normalize on a [128, 256] f32 logits tile."""
    f32 = mybir.dt.float32
    scores_g = scores.rearrange("p (g e) -> p g e", g=G)
    glog = sc_pool.tile([P, G], f32)
    nc.vector.reduce_max(out=glog, in_=scores_g, axis=mybir.AxisListType.X)
    gsort = sc_pool.tile([P, G], f32)
    nc.vector.max(out=gsort, in_=glog)
    # additive mask: 0 for kept groups (>= 4th-largest), -1e30 otherwise
    maskadd = sc_pool.tile([P, G], f32)
    nc.vector.tensor_scalar(
        out=maskadd,
        in0=glog,
        scalar1=gsort[:, TOPK_GROUPS - 1:TOPK_GROUPS],
        scalar2=NEG,
        op0=mybir.AluOpType.is_lt,
        op1=mybir.AluOpType.mult,
    )
    masked = sc_pool.tile([P, E], f32)
    nc.vector.tensor_add(
        masked.rearrange("p (g e) -> p g e", g=G),
        scores_g,
        maskadd.to_broadcast([P, G, EPG]),
    )
    top8 = sc_pool.tile([P, TOPK], f32)
    nc.vector.max(out=top8, in_=masked)
    idx = out_pool.tile([P, TOPK], mybir.dt.uint32)
    nc.vector.max_index(out=idx, in_max=top8, in_values=masked)
    sig = sc_pool.tile([P, TOPK], f32)
    nc.scalar.activation(
        out=sig, in_=top8, func=mybir.ActivationFunctionType.Sigmoid
    )
    ssum = sc_pool.tile([P, 1], f32)
    nc.vector.reduce_sum(out=ssum, in_=sig, axis=mybir.AxisListType.X)
    rec = sc_pool.tile([P, 1], f32)
    nc.vector.reciprocal(out=rec, in_=ssum)
    wres = out_pool.tile([P, TOPK], f32)
    nc.vector.tensor_scalar(
        out=wres,
        in0=sig,
        scalar1=rec[:, 0:1],
        scalar2=ROUTE_SCALE,
        op0=mybir.AluOpType.mult,
        op1=mybir.AluOpType.mult,
    )
    # outputs ride the SWDGE ring so the tiny writes never stall the
    # HWDGE rings that stream x and w
    nc.gpsimd.dma_start(out=wout[t0:t0 + P, :], in_=wres)
    nc.gpsimd.dma_start(out=iout[t0:t0 + P, :], in_=idx)


def _build_fp16fp8():
    nc = bacc.Bacc("TRN2", target_bir_lowering=False, debug=False, num_devices=N_CORES)
    f32 = mybir.dt.float32
    f16 = mybir.dt.float16
    f8 = mybir.dt.float8e4
    # partition-major contiguous host layouts (28.7 KB runs per partition)
    xh = nc.dram_tensor("xh", [P, NB, KC, TB], f16, kind="ExternalInput").ap()
    xl8 = nc.dram_tensor("xl8", [P, NB, KC, TB], f8, kind="ExternalInput").ap()
    wh = nc.dram_tensor("wh", [P, KC, E], f16, kind="ExternalInput").ap()
    w8 = nc.dram_tensor("w8", [P, 2, KC, E], f8, kind="ExternalInput").ap()
    wout = nc.dram_tensor("w_out", [T, TOPK], f32, kind="ExternalOutput").ap()
    iout = nc.dram_tensor("i_out", [T, TOPK], mybir.dt.uint32, kind="ExternalOutput").ap()

    with tile.TileContext(nc) as tc, ExitStack() as ctx:
        wh_pool = ctx.enter_context(tc.tile_pool(name="wh", bufs=1))
        w8_pool = ctx.enter_context(tc.tile_pool(name="w8", bufs=1))
        xh_pool = ctx.enter_context(tc.tile_pool(name="xh", bufs=2))
        x8_pool = ctx.enter_context(tc.tile_pool(name="x8", bufs=2))
        psA_pool = ctx.enter_context(tc.tile_pool(name="psA", bufs=3, space="PSUM"))
        psB_pool = ctx.enter_context(tc.tile_pool(name="psB", bufs=3, space="PSUM"))
        sc_pool = ctx.enter_context(tc.tile_pool(name="scratch", bufs=3))
        out_pool = ctx.enter_context(tc.tile_pool(name="outs", bufs=4))

        # Weights ride the Activation HWDGE ring: no input deps, so they
        # stream from t=0 in parallel with x on the SP ring and never
        # stall it.
        wh_sb, w8_sb = [], []
        for q in range(WQ):
            wtile = wh_pool.tile([P, WCQ, E], f16, tag=f"wh{q}")
            nc.scalar.dma_start(out=wtile, in_=wh[:, q * WCQ:(q + 1) * WCQ, :])
            wh_sb.append(wtile)
        for q in range(WQ):
            w8t = w8_pool.tile([P, 2, WCQ, E], f8, tag=f"w8{q}")
            nc.scalar.dma_start(out=w8t, in_=w8[:, :, q * WCQ:(q + 1) * WCQ, :])
            w8_sb.append(w8t)

        def load_xh(b):
            tiles = []
            for q in range(KQ):
                t_ = xh_pool.tile([P, KCQ, TB], f16, tag=f"xh{q}")
                nc.sync.dma_start(
                    out=t_, in_=xh[:, b, q * KCQ:(q + 1) * KCQ, :]
                )
                tiles.append(t_)
            return tiles

        def load_x8(b, xh_tiles):
            # x8 pair tile [P, 2, KCQ, TB]: [:,0]=fp8(xh) cast on the
            # scalar engine, [:,1]=xl8 DMA'd from host.
            tiles = []
            for q in range(KQ):
                t_ = x8_pool.tile([P, 2, KCQ, TB], f8, tag=f"x8{q}")
                nc.sync.dma_start(
                    out=t_[:, 1], in_=xl8[:, b, q * KCQ:(q + 1) * KCQ, :]
                )
                nc.scalar.activation(
                    out=t_[:, 0], in_=xh_tiles[q],
                    func=mybir.ActivationFunctionType.Copy,
                )
                tiles.append(t_)
            return tiles

        xh_tiles = {0: load_xh(0)}
        x8_tiles = {0: load_x8(0, xh_tiles[0])}

        for b in range(NB):
            if b + 1 < NB:
                xh_tiles[b + 1] = load_xh(b + 1)
                x8_tiles[b + 1] = load_x8(b + 1, xh_tiles[b + 1])
            xh_q = xh_tiles.pop(b)
            x8_q = x8_tiles.pop(b)
            psA_list = []
            for j in range(TPB):
                js = slice(j * P, (j + 1) * P)
                psumA = psA_pool.tile([P, E], f32)
                for k in range(KC):
                    nc.tensor.matmul(
                        psumA,
                        xh_q[k // KCQ][:, k % KCQ, js],
                        wh_sb[k // WCQ][:, k % WCQ, :],
                        start=(k == 0),
                        stop=(k == KC - 1),
                    )
                psA_list.append(psumA)
            for j in range(TPB):
                js = slice(j * P, (j + 1) * P)
                psumB = psB_pool.tile([P, E], f32)
                for k in range(KC):
                    nc.tensor.matmul(
                        psumB,
                        x8_q[k // KCQ][:, :, k % KCQ, js],
                        w8_sb[k // WCQ][:, :, k % WCQ, :],
                        start=(k == 0),
                        stop=(k == KC - 1),
                        perf_mode=mybir.MatmulPerfMode.DoubleRow,
                    )
                scores = sc_pool.tile([P, E], f32)
                nc.scalar.activation(
                    out=scores,
                    in_=psumB,
                    func=mybir.ActivationFunctionType.Copy,
                    scale=CORR_DESCALE,
                )
                nc.vector.tensor_add(scores, scores, psA_list[j])
                _emit_topk(nc, sc_pool, out_pool, scores, wout, iout, b * TB + j * P)
    nc.compile()
    return nc


def _build_fp16x3():
    """Baseline 3-pass fp16 splitting kernel (fallback)."""
    nc = bacc.Bacc("TRN2", target_bir_lowering=False, debug=False, num_devices=N_CORES)
    f32 = mybir.dt.float32
    f16 = mybir.dt.float16
    xh = nc.dram_tensor("xh", [D, T], f16, kind="ExternalInput").ap()
    xl = nc.dram_tensor("xl", [D, T], f16, kind="ExternalInput").ap()
    wh = nc.dram_tensor("wh", [D, E], f16, kind="ExternalInput").ap()
    wl = nc.dram_tensor("wl", [D, E], f16, kind="ExternalInput").ap()
    wout = nc.dram_tensor("w_out", [T, TOPK], f32, kind="ExternalOutput").ap()
    iout = nc.dram_tensor("i_out", [T, TOPK], mybir.dt.uint32, kind="ExternalOutput").ap()

    xh_r = xh.rearrange("(k p) t -> p k t", p=P)
    xl_r = xl.rearrange("(k p) t -> p k t", p=P)
    wh_r = wh.rearrange("(k p) e -> p k e", p=P)
    wl_r = wl.rearrange("(k p) e -> p k e", p=P)

    with tile.TileContext(nc) as tc, ExitStack() as ctx:
        wt_pool = ctx.enter_context(tc.tile_pool(name="wt", bufs=1))
        xt_pool = ctx.enter_context(tc.tile_pool(name="xt", bufs=2))
        psA_pool = ctx.enter_context(tc.tile_pool(name="psA", bufs=4, space="PSUM"))
        psB_pool = ctx.enter_context(tc.tile_pool(name="psB", bufs=4, space="PSUM"))
        sc_pool = ctx.enter_context(tc.tile_pool(name="scratch", bufs=3))
        out_pool = ctx.enter_context(tc.tile_pool(name="outs", bufs=4))

        def load_w(q, which):
            src, lst, tag = (
                (wh_r, wh_sb, f"wh{q}") if which == "h" else (wl_r, wl_sb, f"wl{q}")
            )
            wtile = wt_pool.tile([P, WCQ, E], f16, tag=tag)
            nc.sync.dma_start(out=wtile, in_=src[:, q * WCQ:(q + 1) * WCQ, :])
            lst.append(wtile)

        def load_x_block(b):
            xh_q, xl_q = [], []
            t_lo, t_hi = b * TB, (b + 1) * TB
            for q in range(KQ):
                xtile = xt_pool.tile([P, KCQ, TB], f16, tag=f"xh{q}")
                nc.sync.dma_start(
                    out=xtile, in_=xh_r[:, q * KCQ:(q + 1) * KCQ, t_lo:t_hi]
                )
                xh_q.append(xtile)
                ltile = xt_pool.tile([P, KCQ, TB], f16, tag=f"xl{q}")
                nc.sync.dma_start(
                    out=ltile, in_=xl_r[:, q * KCQ:(q + 1) * KCQ, t_lo:t_hi]
                )
                xl_q.append(ltile)
            return xh_q, xl_q

        wh_sb, wl_sb = [], []
        xh0, xl0 = [], []
        t_hi0 = TB
        for q in range(KQ):
            load_w(2 * q, "h")
            load_w(2 * q + 1, "h")
            xtile = xt_pool.tile([P, KCQ, TB], f16, tag=f"xh{q}")
            nc.sync.dma_start(out=xtile, in_=xh_r[:, q * KCQ:(q + 1) * KCQ, 0:t_hi0])
            xh0.append(xtile)
        for q in range(KQ):
            ltile = xt_pool.tile([P, KCQ, TB], f16, tag=f"xl{q}")
            nc.sync.dma_start(out=ltile, in_=xl_r[:, q * KCQ:(q + 1) * KCQ, 0:t_hi0])
            xl0.append(ltile)
        for q in range(WQ):
            load_w(q, "l")
        blocks = {0: (xh0, xl0)}

        def flush(state):
            bb, xh_q, psA_list, psB_list = state
            for j in range(TPB):
                js = slice(j * P, (j + 1) * P)
                psumB = psB_list[j]
                for k in range(KC):
                    nc.tensor.matmul(
                        psumB,
                        xh_q[k // KCQ][:, k % KCQ, js],
                        wl_sb[k // WCQ][:, k % WCQ, :],
                        start=False,
                        stop=(k == KC - 1),
                    )
                scores = sc_pool.tile([P, E], f32)
                nc.scalar.activation(
                    out=scores,
                    in_=psumB,
                    func=mybir.ActivationFunctionType.Copy,
                    scale=1.0 / XL_SCALE,
                )
                nc.vector.tensor_add(scores, scores, psA_list[j])
                _emit_topk(nc, sc_pool, out_pool, scores, wout, iout, bb * TB + j * P)

        pending = None
        for b in range(NB):
            if b not in blocks:
                blocks[b] = load_x_block(b)
            xh_q, xl_q = blocks.pop(b)
            if b == 0:
                psA_list, psB_list = [], []
                for j in range(TPB):
                    js = slice(j * P, (j + 1) * P)
                    psumA = psA_pool.tile([P, E], f32)
                    for k in range(KC):
                        nc.tensor.matmul(
                            psumA,
                            xh_q[k // KCQ][:, k % KCQ, js],
                            wh_sb[k // WCQ][:, k % WCQ, :],
                            start=(k == 0),
                            stop=(k == KC - 1),
                        )
                    psA_list.append(psumA)
                for j in range(TPB):
                    js = slice(j * P, (j + 1) * P)
                    psumB = psB_pool.tile([P, E], f32)
                    for k in range(KC):
                        nc.tensor.matmul(
                            psumB,
                            xl_q[k // KCQ][:, k % KCQ, js],
                            wh_sb[k // WCQ][:, k % WCQ, :],
                            start=(k == 0),
                            stop=False,
                        )
                    psB_list.append(psumB)
                pending = (b, xh_q, psA_list, psB_list)
                continue
            for j in range(TPB):
                js = slice(j * P, (j + 1) * P)
                psumA = psA_pool.tile([P, E], f32)
                for k in range(KC):
                    nc.tensor.matmul(
                        psumA,
                        xh_q[k // KCQ][:, k % KCQ, js],
                        wh_sb[k // WCQ][:, k % WCQ, :],
                        start=(k == 0),
                        stop=(k == KC - 1),
                    )
                if pending is not None:
                    flush(pending)
                    pending = None
                psumB = psB_pool.tile([P, E], f32)
                for i in range(2 * KC):
                    k = i % KC
                    if i < KC:
                        lhsT = xl_q[k // KCQ][:, k % KCQ, js]
                        rhs = wh_sb[k // WCQ][:, k % WCQ, :]
                    else:
                        lhsT = xh_q[k // KCQ][:, k % KCQ, js]
                        rhs = wl_sb[k // WCQ][:, k % WCQ, :]
                    nc.tensor.matmul(
                        psumB, lhsT, rhs, start=(i == 0), stop=(i == 2 * KC - 1)
                    )
                scores = sc_pool.tile([P, E], f32)
                nc.scalar.activation(
                    out=scores,
                    in_=psumB,
                    func=mybir.ActivationFunctionType.Copy,
                    scale=1.0 / XL_SCALE,
                )
                nc.vector.tensor_add(scores, scores, psumA)
                _emit_topk(nc, sc_pool, out_pool, scores, wout, iout, b * TB + j * P)
    nc.compile()
    return nc


def _get_program(precision):
    key = f"nc_{precision}"
    if key not in _CACHE:
        _CACHE[key] = (
            _build_fp16fp8() if precision == "fp16fp8" else _build_fp16x3()
        )
    return _CACHE[key]


def _pack_x_block_major(a, c):
    """[T_FULL, D] core-c slice -> [P, NB, KC, TB] contiguous."""
    s = a[c * T:(c + 1) * T, :]
    return np.ascontiguousarray(
        s.reshape(NB, TB, KC, P).transpose(3, 0, 2, 1)
    )


def _pack_w(a):
    """[E, D] -> [P, KC, E] contiguous."""
    return np.ascontiguousarray(a.reshape(E, KC, P).transpose(2, 1, 0))


def kernel(x: np.ndarray, weight: np.ndarray, _trace: bool = False, **_kw):
    x = np.asarray(x, dtype=np.float32)
    weight = np.asarray(weight, dtype=np.float32)
    assert x.shape == (T_FULL, D) and weight.shape == (E, D)

    nc = _get_program(PRECISION)
    if PRECISION == "fp16fp8":
        xh16 = x.astype(np.float16)
        xl8_full = ((x - xh16.astype(np.float32)) * np.float32(XL_SCALE)).astype(E4)
        wh16 = weight.astype(np.float16)
        wl8 = ((weight - wh16.astype(np.float32)) * np.float32(WL_SCALE)).astype(E4)
        wh8 = (weight * np.float32(WH8_SCALE)).astype(E4)
        wh_host = _pack_w(wh16)
        w8_host = np.ascontiguousarray(
            np.stack([_pack_w(wl8), _pack_w(wh8)], axis=1)
        )
        in_maps = [
            {
                "xh": _pack_x_block_major(xh16, c),
                "xl8": _pack_x_block_major(xl8_full, c),
                "wh": wh_host,
                "w8": w8_host,
            }
            for c in range(N_CORES)
        ]
    else:
        xt_full = np.ascontiguousarray(x.T)
        wt_host = np.ascontiguousarray(weight.T)
        xh_f, xl_f = xt_full.astype(np.float16), None
        xl_f = ((xt_full - xh_f.astype(np.float32)) * np.float32(XL_SCALE)).astype(
            np.float16
        )
        whh = wt_host.astype(np.float16)
        wll = ((wt_host - whh.astype(np.float32)) * np.float32(XL_SCALE)).astype(
            np.float16
        )
        in_maps = [
            {
                "xh": np.ascontiguousarray(xh_f[:, c * T:(c + 1) * T]),
                "xl": np.ascontiguousarray(xl_f[:, c * T:(c + 1) * T]),
                "wh": whh,
                "wl": wll,
            }
            for c in range(N_CORES)
        ]
    if _trace:
        import tempfile

        res = run_bass_kernel_spmd(
            nc, in_maps, core_ids=list(range(N_CORES)), trace=True,
            tmpdir=tempfile.mkdtemp(prefix="moe_gate_trace_"),
        )
        results = res.results
        _CACHE["last_result"] = {
            "exec_time_ns": res.exec_time_ns,
            "percore": res.mean_exec_time_ns,
            "neff_dir": res.instructions_and_trace[1]
            if res.instructions_and_trace
            else None,
        }
    else:
        res = run_bass_kernel_spmd(nc, in_maps, core_ids=list(range(N_CORES)))
        results = res.results
    w_full = np.concatenate([results[c]["w_out"] for c in range(N_CORES)], axis=0)
    i_full = np.concatenate(
        [results[c]["i_out"].astype(np.int32) for c in range(N_CORES)], axis=0
    )
    return w_full, i_full


# revision 6
# speedup vs baseline: 1.4965x; 1.0411x over previous
"""MoE group-limited routing gate (DeepSeek-style) on 8 Trainium2 NeuronCores.

Computation (per token t over E=256 experts, D=7168 features):
    logits = x @ weight.T                      [T, E]
    group-limited top-k: 8 groups of 32 experts, keep top-4 groups by
    group-max, then top-8 experts among kept groups.
    weights = sigmoid(logits[sel]) normalized to sum 1, * 2.5
Returns (weights [T,8] f32, indices [T,8] int32) like the reference.

Strategy: data-parallel over tokens, 2048 tokens/core, gate weight
replicated.  Matmul precision "fp16fp8":
    logits = xh16 @ wh16  +  2^-17 * (x8 @ wl8 + xl8 @ wh8)
  - main pass: fp16 (11-bit significands, products exact in f32 PSUM)
  - correction: ONE fp8e4m3 DoubleRow pass fusing both residual terms
    (x8 = fp8(xh) cast on-device; xl8 = fp8((x-xh16)*2^11) from host;
    wl8 = fp8((w-wh16)*2^17); wh8 = fp8(w*2^6); descale 2^-17).
    DoubleRow runs fp8 at 2x rate, so the whole correction costs one
    bf16-rate pass -> 2 pass-equivalents total vs fp16x3's 3.
  Host-measured logit err ~1.8e-5 -> idx rel-err ~6e-3 on the graded
  inputs (vs 2e-2 gate).
DMA: x rides the SP HWDGE ring at 3 B/elem (xh16 2B + xl8 1B) in
partition-major contiguous blocks; the small replicated weights ride
the Activation HWDGE ring in parallel so they never serialize the x
stream.  Outputs ride the gpsimd SWDGE ring.
Top-k uses the DVE native max/max_index (top-8 sorted) instructions;
the group top-4 uses a threshold trick (4th-largest group-max) since
sigmoid is monotone and masking is additive on logits.
"""

import numpy as np
import ml_dtypes
from contextlib import ExitStack

import concourse.bacc as bacc
import concourse.tile as tile
from concourse import mybir
from concourse.bass_utils import run_bass_kernel_spmd

N_CORES = 8
T_FULL = 16384
D = 7168
E = 256
G = 8            # expert groups
EPG = E // G     # experts per group = 32
TOPK = 8
TOPK_GROUPS = 4
ROUTE_SCALE = 2.5

P = 128
T = T_FULL // N_CORES       # 2048 tokens per core
KC = D // P                 # 56 contraction chunks
TB = 256                    # tokens per block
NB = T // TB                # 8 blocks
TPB = TB // P               # 2 token-tiles per block
KQ = 4                      # x DMA splits per block
KCQ = KC // KQ              # 14 k-chunks per split
WQ = 8                      # weight DMA splits
WCQ = KC // WQ              # 7 k-chunks per split
NEG = -1.0e30
XL_SCALE = 2.0 ** 11        # xl8 = fp8((x - xh16) * XL_SCALE)
WL_SCALE = 2.0 ** 17        # wl8 = fp8((w - wh16) * WL_SCALE)
WH8_SCALE = 2.0 ** 6        # wh8 = fp8(w * WH8_SCALE); XL_SCALE*WH8_SCALE == WL_SCALE
CORR_DESCALE = 1.0 / WL_SCALE
E4 = ml_dtypes.float8_e4m3
PRECISION = "fp16fp8"       # "fp16fp8" | "fp16x3"

_CACHE = {}


def _emit_topk(nc, sc_pool, out_pool, scores, wout, iout, t0):
    """Group-limited top-k + normalize on a [128, 256] f32 logits tile."""
    f32 = mybir.dt.float32
    scores_g = scores.rearrange("p (g e) -> p g e", g=G)
    glog = sc_pool.tile([P, G], f32)
    nc.vector.reduce_max(out=glog, in_=scores_g, axis=mybir.AxisListType.X)
    gsort = sc_pool.tile([P, G], f32)
    nc.vector.max(out=gsort, in_=glog)
    # additive mask: 0 for kept groups (>= 4th-largest), -1e30 otherwise
    maskadd = sc_pool.tile([P, G], f32)
    nc.vector.tensor_scalar(
        out=maskadd,
        in0=glog,
        scalar1=gsort[:, TOPK_GROUPS - 1:TOPK_GROUPS],
        scalar2=NEG,
        op0=mybir.AluOpType.is_lt,
        op1=mybir.AluOpType.mult,
    )
    masked = sc_pool.tile([P, E], f32)
    nc.vector.tensor_add(
        masked.rearrange("p (g e) -> p g e", g=G),
        scores_g,
        maskadd.to_broadcast([P, G, EPG]),
    )
    top8 = sc_pool.tile([P, TOPK], f32)
    nc.vector.max(out=top8, in_=masked)
    idx = out_pool.tile([P, TOPK], mybir.dt.uint32)
    nc.vector.max_index(out=idx, in_max=top8, in_values=masked)
    sig = sc_pool.tile([P, TOPK], f32)
    nc.scalar.activation(
        out=sig, in_=top8, func=mybir.ActivationFunctionType.Sigmoid
    )
    ssum = sc_pool.tile([P, 1], f32)
    nc.vector.reduce_sum(out=ssum, in_=sig, axis=mybir.AxisListType.X)
    rec = sc_pool.tile([P, 1], f32)
    nc.vector.reciprocal(out=rec, in_=ssum)
    wres = out_pool.tile([P, TOPK], f32)
    nc.vector.tensor_scalar(
        out=wres,
        in0=sig,
        scalar1=rec[:, 0:1],
        scalar2=ROUTE_SCALE,
        op0=mybir.AluOpType.mult,
        op1=mybir.AluOpType.mult,
    )
    # outputs ride the SWDGE ring so the tiny writes never stall the
    # HWDGE rings that stream x and w
    nc.gpsimd.dma_start(out=wout[t0:t0 + P, :], in_=wres)
    nc.gpsimd.dma_start(out=iout[t0:t0 + P, :], in_=idx)


def _build_fp16fp8():
    nc = bacc.Bacc("TRN2", target_bir_lowering=False, debug=False, num_devices=N_CORES)
    f32 = mybir.dt.float32
    f16 = mybir.dt.float16
    f8 = mybir.dt.float8e4
    # partition-major contiguous host layouts (28.7 KB runs per partition)
    xh = nc.dram_tensor("xh", [P, NB, KC, TB], f16, kind="ExternalInput").ap()
    xl8 = nc.dram_tensor("xl8", [P, NB, KC, TB], f8, kind="ExternalInput").ap()
    wh = nc.dram_tensor("wh", [P, KC, E], f16, kind="ExternalInput").ap()
    w8 = nc.dram_tensor("w8", [P, 2, KC, E], f8, kind="ExternalInput").ap()
    wout = nc.dram_tensor("w_out", [T, TOPK], f32, kind="ExternalOutput").ap()
    iout = nc.dram_tensor("i_out", [T, TOPK], mybir.dt.uint32, kind="ExternalOutput").ap()

    with tile.TileContext(nc) as tc, ExitStack() as ctx:
        wh_pool = ctx.enter_context(tc.tile_pool(name="wh", bufs=1))
        w8_pool = ctx.enter_context(tc.tile_pool(name="w8", bufs=1))
        xh_pool = ctx.enter_context(tc.tile_pool(name="xh", bufs=2))
        x8_pool = ctx.enter_context(tc.tile_pool(name="x8", bufs=2))
        psA_pool = ctx.enter_context(tc.tile_pool(name="psA", bufs=4, space="PSUM"))
        psB_pool = ctx.enter_context(tc.tile_pool(name="psB", bufs=4, space="PSUM"))
        sc_pool = ctx.enter_context(tc.tile_pool(name="scratch", bufs=3))
        out_pool = ctx.enter_context(tc.tile_pool(name="outs", bufs=4))

        # Weights ride the Activation HWDGE ring: no input deps, so they
        # stream from t=0 in parallel with x on the SP ring and never
        # stall it.
        wh_sb, w8_sb = [], []
        for q in range(WQ):
            wtile = wh_pool.tile([P, WCQ, E], f16, tag=f"wh{q}")
            nc.scalar.dma_start(out=wtile, in_=wh[:, q * WCQ:(q + 1) * WCQ, :])
            wh_sb.append(wtile)
        for q in range(WQ):
            w8t = w8_pool.tile([P, 2, WCQ, E], f8, tag=f"w8{q}")
            nc.scalar.dma_start(out=w8t, in_=w8[:, :, q * WCQ:(q + 1) * WCQ, :])
            w8_sb.append(w8t)

        def load_xh(b):
            tiles = []
            for q in range(KQ):
                t_ = xh_pool.tile([P, KCQ, TB], f16, tag=f"xh{q}")
                nc.sync.dma_start(
                    out=t_, in_=xh[:, b, q * KCQ:(q + 1) * KCQ, :]
                )
                tiles.append(t_)
            return tiles

        def load_x8(b, xh_tiles):
            # x8 pair tile [P, 2, KCQ, TB]: [:,0]=fp8(xh) cast on-device
            # (quarters alternate scalar/DVE so neither engine saturates),
            # [:,1]=xl8 DMA'd from host.
            tiles = []
            for q in range(KQ):
                t_ = x8_pool.tile([P, 2, KCQ, TB], f8, tag=f"x8{q}")
                nc.sync.dma_start(
                    out=t_[:, 1], in_=xl8[:, b, q * KCQ:(q + 1) * KCQ, :]
                )
                if q % 2 == 0:
                    nc.scalar.activation(
                        out=t_[:, 0], in_=xh_tiles[q],
                        func=mybir.ActivationFunctionType.Copy,
                    )
                else:
                    nc.vector.tensor_copy(out=t_[:, 0], in_=xh_tiles[q])
                tiles.append(t_)
            return tiles

        xh_tiles = {0: load_xh(0)}
        x8_tiles = {0: load_x8(0, xh_tiles[0])}

        for b in range(NB):
            xh_q = xh_tiles.pop(b)
            x8_q = x8_tiles.pop(b)
            psA_list = []
            for j in range(TPB):
                js = slice(j * P, (j + 1) * P)
                psumA = psA_pool.tile([P, E], f32)
                for k in range(KC):
                    nc.tensor.matmul(
                        psumA,
                        xh_q[k // KCQ][:, k % KCQ, js],
                        wh_sb[k // WCQ][:, k % WCQ, :],
                        start=(k == 0),
                        stop=(k == KC - 1),
                    )
                psA_list.append(psumA)
            for j in range(TPB):
                js = slice(j * P, (j + 1) * P)
                psumB = psB_pool.tile([P, E], f32)
                for k in range(KC):
                    nc.tensor.matmul(
                        psumB,
                        x8_q[k // KCQ][:, :, k % KCQ, js],
                        w8_sb[k // WCQ][:, :, k % WCQ, :],
                        start=(k == 0),
                        stop=(k == KC - 1),
                        perf_mode=mybir.MatmulPerfMode.DoubleRow,
                    )
                scores = sc_pool.tile([P, E], f32)
                nc.scalar.activation(
                    out=scores,
                    in_=psumB,
                    func=mybir.ActivationFunctionType.Copy,
                    scale=CORR_DESCALE,
                )
                nc.vector.tensor_add(scores, scores, psA_list[j])
                _emit_topk(nc, sc_pool, out_pool, scores, wout, iout, b * TB + j * P)
            # next block's loads AFTER this block's epilogues: keeps the
            # scalar/DVE queues draining combines+topk (and releasing
            # PSUM) before they start the next casts
            if b + 1 < NB:
                xh_tiles[b + 1] = load_xh(b + 1)
                x8_tiles[b + 1] = load_x8(b + 1, xh_tiles[b + 1])
    nc.compile()
    return nc


def _build_fp16x3():
    """Baseline 3-pass fp16 splitting kernel (fallback)."""
    nc = bacc.Bacc("TRN2", target_bir_lowering=False, debug=False, num_devices=N_CORES)
    f32 = mybir.dt.float32
    f16 = mybir.dt.float16
    xh = nc.dram_tensor("xh", [D, T], f16, kind="ExternalInput").ap()
    xl = nc.dram_tensor("xl", [D, T], f16, kind="ExternalInput").ap()
    wh = nc.dram_tensor("wh", [D, E], f16, kind="ExternalInput").ap()
    wl = nc.dram_tensor("wl", [D, E], f16, kind="ExternalInput").ap()
    wout = nc.dram_tensor("w_out", [T, TOPK], f32, kind="ExternalOutput").ap()
    iout = nc.dram_tensor("i_out", [T, TOPK], mybir.dt.uint32, kind="ExternalOutput").ap()

    xh_r = xh.rearrange("(k p) t -> p k t", p=P)
    xl_r = xl.rearrange("(k p) t -> p k t", p=P)
    wh_r = wh.rearrange("(k p) e -> p k e", p=P)
    wl_r = wl.rearrange("(k p) e -> p k e", p=P)

    with tile.TileContext(nc) as tc, ExitStack() as ctx:
        wt_pool = ctx.enter_context(tc.tile_pool(name="wt", bufs=1))
        xt_pool = ctx.enter_context(tc.tile_pool(name="xt", bufs=2))
        psA_pool = ctx.enter_context(tc.tile_pool(name="psA", bufs=4, space="PSUM"))
        psB_pool = ctx.enter_context(tc.tile_pool(name="psB", bufs=4, space="PSUM"))
        sc_pool = ctx.enter_context(tc.tile_pool(name="scratch", bufs=3))
        out_pool = ctx.enter_context(tc.tile_pool(name="outs", bufs=4))

        def load_w(q, which):
            src, lst, tag = (
                (wh_r, wh_sb, f"wh{q}") if which == "h" else (wl_r, wl_sb, f"wl{q}")
            )
            wtile = wt_pool.tile([P, WCQ, E], f16, tag=tag)
            nc.sync.dma_start(out=wtile, in_=src[:, q * WCQ:(q + 1) * WCQ, :])
            lst.append(wtile)

        def load_x_block(b):
            xh_q, xl_q = [], []
            t_lo, t_hi = b * TB, (b + 1) * TB
            for q in range(KQ):
                xtile = xt_pool.tile([P, KCQ, TB], f16, tag=f"xh{q}")
                nc.sync.dma_start(
                    out=xtile, in_=xh_r[:, q * KCQ:(q + 1) * KCQ, t_lo:t_hi]
                )
                xh_q.append(xtile)
                ltile = xt_pool.tile([P, KCQ, TB], f16, tag=f"xl{q}")
                nc.sync.dma_start(
                    out=ltile, in_=xl_r[:, q * KCQ:(q + 1) * KCQ, t_lo:t_hi]
                )
                xl_q.append(ltile)
            return xh_q, xl_q

        wh_sb, wl_sb = [], []
        xh0, xl0 = [], []
        t_hi0 = TB
        for q in range(KQ):
            load_w(2 * q, "h")
            load_w(2 * q + 1, "h")
            xtile = xt_pool.tile([P, KCQ, TB], f16, tag=f"xh{q}")
            nc.sync.dma_start(out=xtile, in_=xh_r[:, q * KCQ:(q + 1) * KCQ, 0:t_hi0])
            xh0.append(xtile)
        for q in range(KQ):
            ltile = xt_pool.tile([P, KCQ, TB], f16, tag=f"xl{q}")
            nc.sync.dma_start(out=ltile, in_=xl_r[:, q * KCQ:(q + 1) * KCQ, 0:t_hi0])
            xl0.append(ltile)
        for q in range(WQ):
            load_w(q, "l")
        blocks = {0: (xh0, xl0)}

        def flush(state):
            bb, xh_q, psA_list, psB_list = state
            for j in range(TPB):
                js = slice(j * P, (j + 1) * P)
                psumB = psB_list[j]
                for k in range(KC):
                    nc.tensor.matmul(
                        psumB,
                        xh_q[k // KCQ][:, k % KCQ, js],
                        wl_sb[k // WCQ][:, k % WCQ, :],
                        start=False,
                        stop=(k == KC - 1),
                    )
                scores = sc_pool.tile([P, E], f32)
                nc.scalar.activation(
                    out=scores,
                    in_=psumB,
                    func=mybir.ActivationFunctionType.Copy,
                    scale=1.0 / XL_SCALE,
                )
                nc.vector.tensor_add(scores, scores, psA_list[j])
                _emit_topk(nc, sc_pool, out_pool, scores, wout, iout, bb * TB + j * P)

        pending = None
        for b in range(NB):
            if b not in blocks:
                blocks[b] = load_x_block(b)
            xh_q, xl_q = blocks.pop(b)
            if b == 0:
                psA_list, psB_list = [], []
                for j in range(TPB):
                    js = slice(j * P, (j + 1) * P)
                    psumA = psA_pool.tile([P, E], f32)
                    for k in range(KC):
                        nc.tensor.matmul(
                            psumA,
                            xh_q[k // KCQ][:, k % KCQ, js],
                            wh_sb[k // WCQ][:, k % WCQ, :],
                            start=(k == 0),
                            stop=(k == KC - 1),
                        )
                    psA_list.append(psumA)
                for j in range(TPB):
                    js = slice(j * P, (j + 1) * P)
                    psumB = psB_pool.tile([P, E], f32)
                    for k in range(KC):
                        nc.tensor.matmul(
                            psumB,
                            xl_q[k // KCQ][:, k % KCQ, js],
                            wh_sb[k // WCQ][:, k % WCQ, :],
                            start=(k == 0),
                            stop=False,
                        )
                    psB_list.append(psumB)
                pending = (b, xh_q, psA_list, psB_list)
                continue
            for j in range(TPB):
                js = slice(j * P, (j + 1) * P)
                psumA = psA_pool.tile([P, E], f32)
                for k in range(KC):
                    nc.tensor.matmul(
                        psumA,
                        xh_q[k // KCQ][:, k % KCQ, js],
                        wh_sb[k // WCQ][:, k % WCQ, :],
                        start=(k == 0),
                        stop=(k == KC - 1),
                    )
                if pending is not None:
                    flush(pending)
                    pending = None
                psumB = psB_pool.tile([P, E], f32)
                for i in range(2 * KC):
                    k = i % KC
                    if i < KC:
                        lhsT = xl_q[k // KCQ][:, k % KCQ, js]
                        rhs = wh_sb[k // WCQ][:, k % WCQ, :]
                    else:
                        lhsT = xh_q[k // KCQ][:, k % KCQ, js]
                        rhs = wl_sb[k // WCQ][:, k % WCQ, :]
                    nc.tensor.matmul(
                        psumB, lhsT, rhs, start=(i == 0), stop=(i == 2 * KC - 1)
                    )
                scores = sc_pool.tile([P, E], f32)
                nc.scalar.activation(
                    out=scores,
                    in_=psumB,
                    func=mybir.ActivationFunctionType.Copy,
                    scale=1.0 / XL_SCALE,
                )
                nc.vector.tensor_add(scores, scores, psumA)
                _emit_topk(nc, sc_pool, out_pool, scores, wout, iout, b * TB + j * P)
    nc.compile()
    return nc


def _get_program(precision):
    key = f"nc_{precision}"
    if key not in _CACHE:
        _CACHE[key] = (
            _build_fp16fp8() if precision == "fp16fp8" else _build_fp16x3()
        )
    return _CACHE[key]


def _pack_x_block_major(a, c):
    """[T_FULL, D] core-c slice -> [P, NB, KC, TB] contiguous."""
    s = a[c * T:(c + 1) * T, :]
    return np.ascontiguousarray(
        s.reshape(NB, TB, KC, P).transpose(3, 0, 2, 1)
    )


def _pack_w(a):
    """[E, D] -> [P, KC, E] contiguous."""
    return np.ascontiguousarray(a.reshape(E, KC, P).transpose(2, 1, 0))


def kernel(x: np.ndarray, weight: np.ndarray, _trace: bool = False, **_kw):
    x = np.asarray(x, dtype=np.float32)
    weight = np.asarray(weight, dtype=np.float32)
    assert x.shape == (T_FULL, D) and weight.shape == (E, D)

    nc = _get_program(PRECISION)
    if PRECISION == "fp16fp8":
        xh16 = x.astype(np.float16)
        xl8_full = ((x - xh16.astype(np.float32)) * np.float32(XL_SCALE)).astype(E4)
        wh16 = weight.astype(np.float16)
        wl8 = ((weight - wh16.astype(np.float32)) * np.float32(WL_SCALE)).astype(E4)
        wh8 = (weight * np.float32(WH8_SCALE)).astype(E4)
        wh_host = _pack_w(wh16)
        w8_host = np.ascontiguousarray(
            np.stack([_pack_w(wl8), _pack_w(wh8)], axis=1)
        )
        in_maps = [
            {
                "xh": _pack_x_block_major(xh16, c),
                "xl8": _pack_x_block_major(xl8_full, c),
                "wh": wh_host,
                "w8": w8_host,
            }
            for c in range(N_CORES)
        ]
    else:
        xt_full = np.ascontiguousarray(x.T)
        wt_host = np.ascontiguousarray(weight.T)
        xh_f, xl_f = xt_full.astype(np.float16), None
        xl_f = ((xt_full - xh_f.astype(np.float32)) * np.float32(XL_SCALE)).astype(
            np.float16
        )
        whh = wt_host.astype(np.float16)
        wll = ((wt_host - whh.astype(np.float32)) * np.float32(XL_SCALE)).astype(
            np.float16
        )
        in_maps = [
            {
                "xh": np.ascontiguousarray(xh_f[:, c * T:(c + 1) * T]),
                "xl": np.ascontiguousarray(xl_f[:, c * T:(c + 1) * T]),
                "wh": whh,
                "wl": wll,
            }
            for c in range(N_CORES)
        ]
    if _trace:
        import tempfile

        res = run_bass_kernel_spmd(
            nc, in_maps, core_ids=list(range(N_CORES)), trace=True,
            tmpdir=tempfile.mkdtemp(prefix="moe_gate_trace_"),
        )
        results = res.results
        _CACHE["last_result"] = {
            "exec_time_ns": res.exec_time_ns,
            "percore": res.mean_exec_time_ns,
            "neff_dir": res.instructions_and_trace[1]
            if res.instructions_and_trace
            else None,
        }
    else:
        res = run_bass_kernel_spmd(nc, in_maps, core_ids=list(range(N_CORES)))
        results = res.results
    w_full = np.concatenate([results[c]["w_out"] for c in range(N_CORES)], axis=0)
    i_full = np.concatenate(
        [results[c]["i_out"].astype(np.int32) for c in range(N_CORES)], axis=0
    )
    return w_full, i_full


# revision 14
# speedup vs baseline: 1.5141x; 1.0118x over previous
"""MoE group-limited routing gate (DeepSeek-style) on 8 Trainium2 NeuronCores.

Computation (per token t over E=256 experts, D=7168 features):
    logits = x @ weight.T                      [T, E]
    group-limited top-k: 8 groups of 32 experts, keep top-4 groups by
    group-max, then top-8 experts among kept groups.
    weights = sigmoid(logits[sel]) normalized to sum 1, * 2.5
Returns (weights [T,8] f32, indices [T,8] int32) like the reference.

Strategy: data-parallel over tokens, 2048 tokens/core, gate weight
replicated.  Matmul precision "fp16fp8":
    logits = xh16 @ wh16  +  2^-17 * (x8 @ wl8 + xl8 @ wh8)
  - main pass: fp16 (11-bit significands, products exact in f32 PSUM)
  - correction: ONE fp8e4m3 DoubleRow pass fusing both residual terms
    (x8 = fp8(xh) cast on-device; xl8 = fp8((x-xh16)*2^11) from host;
    wl8 = fp8((w-wh16)*2^17); wh8 = fp8(w*2^6); descale 2^-17).
    DoubleRow runs fp8 at 2x rate, so the whole correction costs one
    bf16-rate pass -> 2 pass-equivalents total vs fp16x3's 3.
  Host-measured logit err ~1.8e-5 -> idx rel-err ~6e-3 on the graded
  inputs (vs 2e-2 gate).
DMA: x rides the SP HWDGE ring at 3 B/elem (xh16 2B + xl8 1B) in
partition-major contiguous blocks; the small replicated weights ride
the Activation HWDGE ring in parallel so they never serialize the x
stream.  Outputs ride the gpsimd SWDGE ring.
Top-k uses the DVE native max/max_index (top-8 sorted) instructions;
the group top-4 uses a threshold trick (4th-largest group-max) since
sigmoid is monotone and masking is additive on logits.
"""

import numpy as np
import ml_dtypes
from contextlib import ExitStack

import concourse.bacc as bacc
import concourse.tile as tile
from concourse import mybir
from concourse.bass_utils import run_bass_kernel_spmd

N_CORES = 8
T_FULL = 16384
D = 7168
E = 256
G = 8            # expert groups
EPG = E // G     # experts per group = 32
TOPK = 8
TOPK_GROUPS = 4
ROUTE_SCALE = 2.5

P = 128
T = T_FULL // N_CORES       # 2048 tokens per core
KC = D // P                 # 56 contraction chunks
TB = 256                    # tokens per block
NB = T // TB                # 8 blocks
TPB = TB // P               # 2 token-tiles per block
KQ = 4                      # x DMA splits per block
KCQ = KC // KQ              # 14 k-chunks per split
WQ = 8                      # weight DMA splits
WCQ = KC // WQ              # 7 k-chunks per split
NEG = -1.0e30
XL_SCALE = 2.0 ** 11        # xl8 = fp8((x - xh16) * XL_SCALE)
W_SCALE = 2.0 ** 6          # wh16 stores w * 2^6 (exact power-of-2 rescale) so
                            # that wh8 = fp8(wh16) needs no scale on the cast;
                            # the 2^-6 is folded into the sigmoid's scale arg
CORR_DESCALE = 1.0 / XL_SCALE
E4 = ml_dtypes.float8_e4m3
PRECISION = "fp16fp8"       # "fp16fp8" | "fp16x3"

_CACHE = {}


def _emit_topk(nc, sc_pool, out_pool, scores, wout, iout, t0,
               sig_scale=1.0, out_eng=None):
    """Group-limited top-k + normalize on a [128, 256] f32 logits tile.

    ``scores`` may be pre-scaled logits (monotone, so group-mask and top-k
    are unaffected); ``sig_scale`` restores true logits inside the sigmoid.
    """
    f32 = mybir.dt.float32
    scores_g = scores.rearrange("p (g e) -> p g e", g=G)
    glog = sc_pool.tile([P, G], f32)
    nc.vector.reduce_max(out=glog, in_=scores_g, axis=mybir.AxisListType.X)
    gsort = sc_pool.tile([P, G], f32)
    nc.vector.max(out=gsort, in_=glog)
    # additive mask: 0 for kept groups (>= 4th-largest), -1e30 otherwise
    maskadd = sc_pool.tile([P, G], f32)
    nc.vector.tensor_scalar(
        out=maskadd,
        in0=glog,
        scalar1=gsort[:, TOPK_GROUPS - 1:TOPK_GROUPS],
        scalar2=NEG,
        op0=mybir.AluOpType.is_lt,
        op1=mybir.AluOpType.mult,
    )
    masked = sc_pool.tile([P, E], f32)
    nc.vector.tensor_add(
        masked.rearrange("p (g e) -> p g e", g=G),
        scores_g,
        maskadd.to_broadcast([P, G, EPG]),
    )
    top8 = sc_pool.tile([P, TOPK], f32)
    nc.vector.max(out=top8, in_=masked)
    idx = out_pool.tile([P, TOPK], mybir.dt.uint32)
    nc.vector.max_index(out=idx, in_max=top8, in_values=masked)
    sig = sc_pool.tile([P, TOPK], f32)
    nc.scalar.activation(
        out=sig, in_=top8, func=mybir.ActivationFunctionType.Sigmoid,
        scale=sig_scale,
    )
    ssum = sc_pool.tile([P, 1], f32)
    nc.vector.reduce_sum(out=ssum, in_=sig, axis=mybir.AxisListType.X)
    rec = sc_pool.tile([P, 1], f32)
    nc.vector.reciprocal(out=rec, in_=ssum)
    wres = out_pool.tile([P, TOPK], f32)
    nc.vector.tensor_scalar(
        out=wres,
        in0=sig,
        scalar1=rec[:, 0:1],
        scalar2=ROUTE_SCALE,
        op0=mybir.AluOpType.mult,
        op1=mybir.AluOpType.mult,
    )
    # outputs ride the SWDGE ring so the tiny writes never stall the
    # HWDGE rings that stream x and w; the tail blocks ride the (by
    # then idle) sync ring instead, which drains faster
    eng = out_eng if out_eng is not None else nc.gpsimd
    eng.dma_start(out=wout[t0:t0 + P, :], in_=wres)
    eng.dma_start(out=iout[t0:t0 + P, :], in_=idx)


def _build_fp16fp8():
    nc = bacc.Bacc("TRN2", target_bir_lowering=False, debug=False, num_devices=N_CORES)
    f32 = mybir.dt.float32
    f16 = mybir.dt.float16
    f8 = mybir.dt.float8e4
    # partition-major contiguous host layouts (28.7 KB runs per partition)
    xh = nc.dram_tensor("xh", [P, NB, KC, TB], f16, kind="ExternalInput").ap()
    xl8 = nc.dram_tensor("xl8", [P, NB, KC, TB], f8, kind="ExternalInput").ap()
    wh = nc.dram_tensor("wh", [P, KC, E], f16, kind="ExternalInput").ap()
    wl8d = nc.dram_tensor("wl8", [P, KC, E], f8, kind="ExternalInput").ap()
    wout = nc.dram_tensor("w_out", [T, TOPK], f32, kind="ExternalOutput").ap()
    iout = nc.dram_tensor("i_out", [T, TOPK], mybir.dt.uint32, kind="ExternalOutput").ap()

    with tile.TileContext(nc) as tc, ExitStack() as ctx:
        wh_pool = ctx.enter_context(tc.tile_pool(name="wh", bufs=1))
        w8_pool = ctx.enter_context(tc.tile_pool(name="w8", bufs=1))
        xh_pool = ctx.enter_context(tc.tile_pool(name="xh", bufs=2))
        x8_pool = ctx.enter_context(tc.tile_pool(name="x8", bufs=2))
        psA_pool = ctx.enter_context(tc.tile_pool(name="psA", bufs=4, space="PSUM"))
        psB_pool = ctx.enter_context(tc.tile_pool(name="psB", bufs=4, space="PSUM"))
        sc_pool = ctx.enter_context(tc.tile_pool(name="scratch", bufs=3))
        out_pool = ctx.enter_context(tc.tile_pool(name="outs", bufs=4))

        # Weights ride the Activation HWDGE ring: no input deps, so they
        # stream from t=0 in parallel with x on the SP ring and never
        # stall it.  Only wh (fp16, pre-scaled by 2^6) and wl8 come from
        # HBM; wh8 = fp8(wh) is a plain on-device cast (no scale needed
        # thanks to the 2^6 pre-scale), halving the w8 head bytes.
        wh_sb, w8_sb = [], []
        for q in range(WQ):
            wtile = wh_pool.tile([P, WCQ, E], f16, tag=f"wh{q}")
            nc.scalar.dma_start(out=wtile, in_=wh[:, q * WCQ:(q + 1) * WCQ, :])
            wh_sb.append(wtile)
        for q in range(WQ):
            w8t = w8_pool.tile([P, 2, WCQ, E], f8, tag=f"w8{q}")
            nc.scalar.dma_start(
                out=w8t[:, 0], in_=wl8d[:, q * WCQ:(q + 1) * WCQ, :]
            )
            if q % 2 == 0:
                nc.scalar.activation(
                    out=w8t[:, 1], in_=wh_sb[q],
                    func=mybir.ActivationFunctionType.Copy,
                )
            else:
                nc.vector.tensor_copy(out=w8t[:, 1], in_=wh_sb[q])
            w8_sb.append(w8t)

        def load_xh(b):
            tiles = []
            for q in range(KQ):
                t_ = xh_pool.tile([P, KCQ, TB], f16, tag=f"xh{q}")
                nc.sync.dma_start(
                    out=t_, in_=xh[:, b, q * KCQ:(q + 1) * KCQ, :]
                )
                tiles.append(t_)
            return tiles

        def load_x8(b, xh_tiles):
            # x8 pair tile [P, 2, KCQ, TB]: [:,0]=fp8(xh) cast on-device
            # (quarters alternate scalar/DVE so neither engine saturates),
            # [:,1]=xl8 DMA'd from host.
            tiles = []
            for q in range(KQ):
                t_ = x8_pool.tile([P, 2, KCQ, TB], f8, tag=f"x8{q}")
                nc.sync.dma_start(
                    out=t_[:, 1], in_=xl8[:, b, q * KCQ:(q + 1) * KCQ, :]
                )
                if q % 2 == 0:
                    nc.scalar.activation(
                        out=t_[:, 0], in_=xh_tiles[q],
                        func=mybir.ActivationFunctionType.Copy,
                    )
                else:
                    nc.vector.tensor_copy(out=t_[:, 0], in_=xh_tiles[q])
                tiles.append(t_)
            return tiles

        xh_tiles = {0: load_xh(0)}
        x8_tiles = {0: load_x8(0, xh_tiles[0])}

        for b in range(NB):
            xh_q = xh_tiles.pop(b)
            x8_q = x8_tiles.pop(b)
            psA_list = []
            for j in range(TPB):
                js = slice(j * P, (j + 1) * P)
                psumA = psA_pool.tile([P, E], f32)
                for k in range(KC):
                    nc.tensor.matmul(
                        psumA,
                        xh_q[k // KCQ][:, k % KCQ, js],
                        wh_sb[k // WCQ][:, k % WCQ, :],
                        start=(k == 0),
                        stop=(k == KC - 1),
                    )
                psA_list.append(psumA)
            for j in range(TPB):
                js = slice(j * P, (j + 1) * P)
                psumB = psB_pool.tile([P, E], f32)
                for k in range(KC):
                    nc.tensor.matmul(
                        psumB,
                        x8_q[k // KCQ][:, :, k % KCQ, js],
                        w8_sb[k // WCQ][:, :, k % WCQ, :],
                        start=(k == 0),
                        stop=(k == KC - 1),
                        perf_mode=mybir.MatmulPerfMode.DoubleRow,
                    )
                scores = sc_pool.tile([P, E], f32)
                nc.scalar.activation(
                    out=scores,
                    in_=psumB,
                    func=mybir.ActivationFunctionType.Copy,
                    scale=CORR_DESCALE,
                )
                nc.vector.tensor_add(scores, scores, psA_list[j])
                _emit_topk(
                    nc, sc_pool, out_pool, scores, wout, iout, b * TB + j * P,
                    sig_scale=1.0 / W_SCALE,
                    out_eng=nc.sync if b >= NB - 2 else None,
                )
            # next block's loads AFTER this block's epilogues: keeps the
            # scalar/DVE queues draining combines+topk (and releasing
            # PSUM) before they start the next casts
            if b + 1 < NB:
                xh_tiles[b + 1] = load_xh(b + 1)
                x8_tiles[b + 1] = load_x8(b + 1, xh_tiles[b + 1])
    nc.compile()
    return nc


def _build_fp16x3():
    """Baseline 3-pass fp16 splitting kernel (fallback)."""
    nc = bacc.Bacc("TRN2", target_bir_lowering=False, debug=False, num_devices=N_CORES)
    f32 = mybir.dt.float32
    f16 = mybir.dt.float16
    xh = nc.dram_tensor("xh", [D, T], f16, kind="ExternalInput").ap()
    xl = nc.dram_tensor("xl", [D, T], f16, kind="ExternalInput").ap()
    wh = nc.dram_tensor("wh", [D, E], f16, kind="ExternalInput").ap()
    wl = nc.dram_tensor("wl", [D, E], f16, kind="ExternalInput").ap()
    wout = nc.dram_tensor("w_out", [T, TOPK], f32, kind="ExternalOutput").ap()
    iout = nc.dram_tensor("i_out", [T, TOPK], mybir.dt.uint32, kind="ExternalOutput").ap()

    xh_r = xh.rearrange("(k p) t -> p k t", p=P)
    xl_r = xl.rearrange("(k p) t -> p k t", p=P)
    wh_r = wh.rearrange("(k p) e -> p k e", p=P)
    wl_r = wl.rearrange("(k p) e -> p k e", p=P)

    with tile.TileContext(nc) as tc, ExitStack() as ctx:
        wt_pool = ctx.enter_context(tc.tile_pool(name="wt", bufs=1))
        xt_pool = ctx.enter_context(tc.tile_pool(name="xt", bufs=2))
        psA_pool = ctx.enter_context(tc.tile_pool(name="psA", bufs=4, space="PSUM"))
        psB_pool = ctx.enter_context(tc.tile_pool(name="psB", bufs=4, space="PSUM"))
        sc_pool = ctx.enter_context(tc.tile_pool(name="scratch", bufs=3))
        out_pool = ctx.enter_context(tc.tile_pool(name="outs", bufs=4))

        def load_w(q, which):
            src, lst, tag = (
                (wh_r, wh_sb, f"wh{q}") if which == "h" else (wl_r, wl_sb, f"wl{q}")
            )
            wtile = wt_pool.tile([P, WCQ, E], f16, tag=tag)
            nc.sync.dma_start(out=wtile, in_=src[:, q * WCQ:(q + 1) * WCQ, :])
            lst.append(wtile)

        def load_x_block(b):
            xh_q, xl_q = [], []
            t_lo, t_hi = b * TB, (b + 1) * TB
            for q in range(KQ):
                xtile = xt_pool.tile([P, KCQ, TB], f16, tag=f"xh{q}")
                nc.sync.dma_start(
                    out=xtile, in_=xh_r[:, q * KCQ:(q + 1) * KCQ, t_lo:t_hi]
                )
                xh_q.append(xtile)
                ltile = xt_pool.tile([P, KCQ, TB], f16, tag=f"xl{q}")
                nc.sync.dma_start(
                    out=ltile, in_=xl_r[:, q * KCQ:(q + 1) * KCQ, t_lo:t_hi]
                )
                xl_q.append(ltile)
            return xh_q, xl_q

        wh_sb, wl_sb = [], []
        xh0, xl0 = [], []
        t_hi0 = TB
        for q in range(KQ):
            load_w(2 * q, "h")
            load_w(2 * q + 1, "h")
            xtile = xt_pool.tile([P, KCQ, TB], f16, tag=f"xh{q}")
            nc.sync.dma_start(out=xtile, in_=xh_r[:, q * KCQ:(q + 1) * KCQ, 0:t_hi0])
            xh0.append(xtile)
        for q in range(KQ):
            ltile = xt_pool.tile([P, KCQ, TB], f16, tag=f"xl{q}")
            nc.sync.dma_start(out=ltile, in_=xl_r[:, q * KCQ:(q + 1) * KCQ, 0:t_hi0])
            xl0.append(ltile)
        for q in range(WQ):
            load_w(q, "l")
        blocks = {0: (xh0, xl0)}

        def flush(state):
            bb, xh_q, psA_list, psB_list = state
            for j in range(TPB):
                js = slice(j * P, (j + 1) * P)
                psumB = psB_list[j]
                for k in range(KC):
                    nc.tensor.matmul(
                        psumB,
                        xh_q[k // KCQ][:, k % KCQ, js],
                        wl_sb[k // WCQ][:, k % WCQ, :],
                        start=False,
                        stop=(k == KC - 1),
                    )
                scores = sc_pool.tile([P, E], f32)
                nc.scalar.activation(
                    out=scores,
                    in_=psumB,
                    func=mybir.ActivationFunctionType.Copy,
                    scale=1.0 / XL_SCALE,
                )
                nc.vector.tensor_add(scores, scores, psA_list[j])
                _emit_topk(nc, sc_pool, out_pool, scores, wout, iout, bb * TB + j * P)

        pending = None
        for b in range(NB):
            if b not in blocks:
                blocks[b] = load_x_block(b)
            xh_q, xl_q = blocks.pop(b)
            if b == 0:
                psA_list, psB_list = [], []
                for j in range(TPB):
                    js = slice(j * P, (j + 1) * P)
                    psumA = psA_pool.tile([P, E], f32)
                    for k in range(KC):
                        nc.tensor.matmul(
                            psumA,
                            xh_q[k // KCQ][:, k % KCQ, js],
                            wh_sb[k // WCQ][:, k % WCQ, :],
                            start=(k == 0),
                            stop=(k == KC - 1),
                        )
                    psA_list.append(psumA)
                for j in range(TPB):
                    js = slice(j * P, (j + 1) * P)
                    psumB = psB_pool.tile([P, E], f32)
                    for k in range(KC):
                        nc.tensor.matmul(
                            psumB,
                            xl_q[k // KCQ][:, k % KCQ, js],
                            wh_sb[k // WCQ][:, k % WCQ, :],
                            start=(k == 0),
                            stop=False,
                        )
                    psB_list.append(psumB)
                pending = (b, xh_q, psA_list, psB_list)
                continue
            for j in range(TPB):
                js = slice(j * P, (j + 1) * P)
                psumA = psA_pool.tile([P, E], f32)
                for k in range(KC):
                    nc.tensor.matmul(
                        psumA,
                        xh_q[k // KCQ][:, k % KCQ, js],
                        wh_sb[k // WCQ][:, k % WCQ, :],
                        start=(k == 0),
                        stop=(k == KC - 1),
                    )
                if pending is not None:
                    flush(pending)
                    pending = None
                psumB = psB_pool.tile([P, E], f32)
                for i in range(2 * KC):
                    k = i % KC
                    if i < KC:
                        lhsT = xl_q[k // KCQ][:, k % KCQ, js]
                        rhs = wh_sb[k // WCQ][:, k % WCQ, :]
                    else:
                        lhsT = xh_q[k // KCQ][:, k % KCQ, js]
                        rhs = wl_sb[k // WCQ][:, k % WCQ, :]
                    nc.tensor.matmul(
                        psumB, lhsT, rhs, start=(i == 0), stop=(i == 2 * KC - 1)
                    )
                scores = sc_pool.tile([P, E], f32)
                nc.scalar.activation(
                    out=scores,
                    in_=psumB,
                    func=mybir.ActivationFunctionType.Copy,
                    scale=1.0 / XL_SCALE,
                )
                nc.vector.tensor_add(scores, scores, psumA)
                _emit_topk(nc, sc_pool, out_pool, scores, wout, iout, b * TB + j * P)
    nc.compile()
    return nc


def _get_program(precision):
    key = f"nc_{precision}"
    if key not in _CACHE:
        _CACHE[key] = (
            _build_fp16fp8() if precision == "fp16fp8" else _build_fp16x3()
        )
    return _CACHE[key]


def _pack_x_block_major(a, c):
    """[T_FULL, D] core-c slice -> [P, NB, KC, TB] contiguous."""
    s = a[c * T:(c + 1) * T, :]
    return np.ascontiguousarray(
        s.reshape(NB, TB, KC, P).transpose(3, 0, 2, 1)
    )


def _pack_w(a):
    """[E, D] -> [P, KC, E] contiguous."""
    return np.ascontiguousarray(a.reshape(E, KC, P).transpose(2, 1, 0))


def kernel(x: np.ndarray, weight: np.ndarray, _trace: bool = False, **_kw):
    x = np.asarray(x, dtype=np.float32)
    weight = np.asarray(weight, dtype=np.float32)
    assert x.shape == (T_FULL, D) and weight.shape == (E, D)

    nc = _get_program(PRECISION)
    if PRECISION == "fp16fp8":
        xh16 = x.astype(np.float16)
        xl8_full = ((x - xh16.astype(np.float32)) * np.float32(XL_SCALE)).astype(E4)
        ws = weight * np.float32(W_SCALE)
        wh16 = ws.astype(np.float16)
        wl8 = ((ws - wh16.astype(np.float32)) * np.float32(XL_SCALE)).astype(E4)
        wh_host = _pack_w(wh16)
        wl8_host = _pack_w(wl8)
        in_maps = [
            {
                "xh": _pack_x_block_major(xh16, c),
                "xl8": _pack_x_block_major(xl8_full, c),
                "wh": wh_host,
                "wl8": wl8_host,
            }
            for c in range(N_CORES)
        ]
    else:
        xt_full = np.ascontiguousarray(x.T)
        wt_host = np.ascontiguousarray(weight.T)
        xh_f, xl_f = xt_full.astype(np.float16), None
        xl_f = ((xt_full - xh_f.astype(np.float32)) * np.float32(XL_SCALE)).astype(
            np.float16
        )
        whh = wt_host.astype(np.float16)
        wll = ((wt_host - whh.astype(np.float32)) * np.float32(XL_SCALE)).astype(
            np.float16
        )
        in_maps = [
            {
                "xh": np.ascontiguousarray(xh_f[:, c * T:(c + 1) * T]),
                "xl": np.ascontiguousarray(xl_f[:, c * T:(c + 1) * T]),
                "wh": whh,
                "wl": wll,
            }
            for c in range(N_CORES)
        ]
    if _trace:
        import tempfile

        res = run_bass_kernel_spmd(
            nc, in_maps, core_ids=list(range(N_CORES)), trace=True,
            tmpdir=tempfile.mkdtemp(prefix="moe_gate_trace_"),
        )
        results = res.results
        _CACHE["last_result"] = {
            "exec_time_ns": res.exec_time_ns,
            "percore": res.mean_exec_time_ns,
            "neff_dir": res.instructions_and_trace[1]
            if res.instructions_and_trace
            else None,
        }
    else:
        res = run_bass_kernel_spmd(nc, in_maps, core_ids=list(range(N_CORES)))
        results = res.results
    w_full = np.concatenate([results[c]["w_out"] for c in range(N_CORES)], axis=0)
    i_full = np.concatenate(
        [results[c]["i_out"].astype(np.int32) for c in range(N_CORES)], axis=0
    )
    return w_full, i_full
